# revision 1
# baseline (speedup 1.0000x reference)
"""CPC / NT-Xent loss kernel for 8 Trainium2 NeuronCores.

Reference computation (x, y: [8192, 256] f32):
    x_norm, y_norm = L2-normalized rows
    xy = concat(x_norm, y_norm)            # [16384, 256]
    sim = xy @ xy.T                        # [16384, 16384]
    denom_i = sum_j exp(sim_ij / tau) - exp(sim_ii / tau)
    pos_i   = dot(xy_i, yx_i)  (yx = concat(y_norm, x_norm))
    loss = mean( log(denom_i) - pos_i / tau )

Sharding: the 2N=16384 rows are data-parallel across the 8 cores.  Each
core receives the full row matrix ROTATED so its own 2048 rows sit at
local rows 0..2048 — the kernel is then a single SPMD program with no
core-dependent control flow.  The partner row (for pos_i) of local row i
is always local row 8192+i, independent of the rotation.

Per core, fully fused on-chip (the 16384x16384 sim matrix never touches
HBM):
  - load b [16384, 256] f32, row tiles [128, 256]
  - ss = row sums of squares (DVE scalar_tensor_tensor accum)
  - inv = rsqrt(ss) via DVE-only Newton iteration (no activation-table
    switches; the scalar engine keeps the Exp table loaded throughout)
  - rows scaled to unit norm and cast to bf16 (DVE), PE-transposed into
    PSUM collectors, copied to SBUF as bf16 B_T [256, 16384]
  - Gram row-block: for each m-tile (128 rows) x 2048-col chunk:
    bf16 matmul (K=256 = 2 accum steps) -> PSUM f32
  - ACT exp(2*psum) with accum_out giving the row-chunk sums directly;
    the (dead) exp values are written as bf16 to SBUF, which engages the
    scalar engine's 2x packed-output mode — measured faster than f32
    in-place, and keeps ACT ahead of PE so the PE never micro-idles
    (HAM stays warm)
  - denominator = rowsum - e^2  (sim_ii == 1 exactly)
  - nt_xent = ln(denominator) - 2*pos ; written out per row
The next group's load/normalize/transpose work is emitted interleaved
with the current group's matmuls so PSUM slot recycling overlaps with
compute instead of stalling at group boundaries.
Host: concatenates the 8 x 2048 per-row losses and takes the mean.
"""

import numpy as np
from contextlib import ExitStack

import concourse.bacc as bacc
import concourse.bass as bass
import concourse.tile as tile
import concourse.mybir as mybir
from concourse import bass_utils
from concourse.masks import make_identity

F32 = mybir.dt.float32
BF16 = mybir.dt.bfloat16
AF = mybir.ActivationFunctionType
ALU = mybir.AluOpType

P = 128          # partitions
TAU = 0.5
N_CORES = 8

# Full-problem geometry (hardcoded per contract)
B_ROWS = 8192    # rows in x (and y)
H = 256          # feature dim (= 2 k-tiles of 128)
N_TOTAL = 2 * B_ROWS          # 16384 rows of the concat matrix
N_MINE = N_TOTAL // N_CORES   # 2048 rows per core
CHUNK = 2048                  # columns processed per outer step (4 PSUM banks f32)

NEWTON_ITERS = 5              # rsqrt Newton steps after constant seed


class _Ctx:
    """Bag of state shared by the emission helpers."""


def build_program(n_total=N_TOTAL, n_mine=N_MINE, chunk=CHUNK, repeat=1,
                  nt_at=8, ld_at=10, exp_sbuf=True, enable_asserts=False):
    """Build the SPMD Bass program. Returns (nc, in_name, out_name).

    repeat>1 re-runs the whole computation sequentially (same math, its
    own output slice) — used to measure device time differentially when
    NTFF tracing is unavailable.
    """
    T = n_total // P              # total row tiles
    MT = n_mine // P              # my row tiles (M dimension)
    TPG = chunk // P              # row tiles ingested per outer step
    G = n_total // chunk          # outer steps
    NJ = chunk // 512             # 512-wide matmul slices per chunk
    half = T // 2                 # partner offset, in tiles
    assert H == 2 * P and half >= MT and chunk % 512 == 0
    assert n_total % chunk == 0 and n_mine % P == 0
    assert MT * P <= chunk        # lhsT slices live in the group-0 BT tile

    nc = bacc.Bacc(
        "TRN2",
        target_bir_lowering=False,
        debug=False,
        enable_asserts=enable_asserts,
        num_devices=N_CORES,
    )
    b_dram = nc.dram_tensor("b", [n_total, H], F32, kind="ExternalInput")
    # one output slice per repeat so no rep is dead code
    nt_dram = nc.dram_tensor("nt", [P, MT * repeat], F32, kind="ExternalOutput")

    with ExitStack() as ctx:
        tc = ctx.enter_context(tile.TileContext(nc))

        c = _Ctx()
        c.nc, c.b_ap, c.nt_dram = nc, b_dram.ap(), nt_dram
        c.T, c.MT, c.TPG, c.G, c.NJ, c.half, c.chunk = T, MT, TPG, G, NJ, half, chunk
        c.NT_AT, c.LD_AT = min(nt_at, MT - 2), min(ld_at, MT - 1)
        c.exp_sbuf = exp_sbuf

        c.const_pool = ctx.enter_context(tc.tile_pool(name="const", bufs=1))
        bt_pool = ctx.enter_context(tc.tile_pool(name="bt", bufs=1))
        stat_pool = ctx.enter_context(tc.tile_pool(name="stat", bufs=1))
        c.load_pool = ctx.enter_context(tc.tile_pool(name="load", bufs=3))
        c.keep_pool = ctx.enter_context(tc.tile_pool(name="keep", bufs=MT))
        c.nrm_pool = ctx.enter_context(tc.tile_pool(name="nrm", bufs=TPG + 2))
        c.sq_pool = ctx.enter_context(tc.tile_pool(name="sq", bufs=4))
        c.nwt_pool = ctx.enter_context(tc.tile_pool(name="nwt", bufs=4))
        c.exp_pool = ctx.enter_context(tc.tile_pool(name="expo", bufs=3))
        c.psum_pool = ctx.enter_context(tc.tile_pool(name="ps", bufs=2, space="PSUM"))
        c.fin_pool = ctx.enter_context(tc.tile_pool(name="fin", bufs=1))

        c.identity = c.const_pool.tile([P, P], BF16)
        make_identity(nc, c.identity[:])

        # B_T: normalized rows, transposed, bf16.  Two k-halves, one tile
        # per column group (separate tiles -> no false cross-group deps).
        c.BT0s = [bt_pool.tile([P, chunk], BF16, tag=f"bt0_{g}", name=f"bt0_{g}")
                  for g in range(G)]
        c.BT1s = [bt_pool.tile([P, chunk], BF16, tag=f"bt1_{g}", name=f"bt1_{g}")
                  for g in range(G)]

        c.ss_all = stat_pool.tile([P, T], F32)     # row sums of squares
        c.inv_all = stat_pool.tile([P, T], F32)    # 1/norm
        c.rs_all = stat_pool.tile([P, MT * G], F32)  # exp sums, col = m*G+g
        c.pos_all = stat_pool.tile([P, MT], F32)   # pos_sim per my-row

        for rep in range(repeat):
            emit_rep(c, rep)

    nc.compile()
    return nc, "b", "nt"


def emit_loads(c, g):
    """DMA the whole group as ONE slab + row sums of squares + Newton
    rsqrt.  A single 2MB DMA instead of 16 x 128KB cuts per-DMA queue
    overhead and sequencer/semaphore traffic."""
    nc = c.nc
    slab = c.load_pool.tile([P, c.TPG, H], F32, tag="raw", name=f"slab_{g}")
    src = c.b_ap[g * c.TPG * P:(g + 1) * c.TPG * P, :].rearrange(
        "(t p) m -> p t m", p=P)
    nc.sync.dma_start(out=slab[:], in_=src)
    raws = []
    for t in range(c.TPG):
        ti = g * c.TPG + t
        raw = slab[:, t, :]
        sq = c.sq_pool.tile([P, H], F32, tag="sq", name="sqs")
        nc.vector.scalar_tensor_tensor(
            out=sq[:], in0=raw, scalar=1.0, in1=raw,
            op0=ALU.mult, op1=ALU.mult,
            accum_out=c.ss_all[:, ti:ti + 1],
        )
        raws.append(raw)
    c.raws[g] = raws

    # inv = rsqrt(ss), DVE-only Newton iteration.
    # seed y0 = H**-0.5 (ss concentrates near H for unit-variance rows);
    # y1 = y0*(1.5 - 0.5*y0^2*ss) folds into one tensor_scalar op.
    u = c.ss_all[:, g * c.TPG:(g + 1) * c.TPG]
    y0 = float(H) ** -0.5
    y = c.nwt_pool.tile([P, c.TPG], F32, tag="nwty", name="nwty")
    nc.vector.tensor_scalar(
        out=y[:], in0=u, scalar1=-0.5 * y0 ** 3, scalar2=1.5 * y0,
        op0=ALU.mult, op1=ALU.add)
    inv_slice = c.inv_all[:, g * c.TPG:(g + 1) * c.TPG]
    for it in range(NEWTON_ITERS - 1):
        t1 = c.nwt_pool.tile([P, c.TPG], F32, tag="nwtt", name="nwtt")
        nc.vector.scalar_tensor_tensor(
            out=t1[:], in0=y[:], scalar=1.0, in1=y[:],
            op0=ALU.mult, op1=ALU.mult)               # y^2
        t2 = c.nwt_pool.tile([P, c.TPG], F32, tag="nwtt2", name="nwtt2")
        nc.vector.scalar_tensor_tensor(
            out=t2[:], in0=u, scalar=-0.5, in1=t1[:],
            op0=ALU.mult, op1=ALU.mult)               # -0.5*ss*y^2
        last = it == NEWTON_ITERS - 2
        ynew = inv_slice if last else c.nwt_pool.tile(
            [P, c.TPG], F32, tag="nwty", name="nwty")
        nc.vector.scalar_tensor_tensor(
            out=ynew if last else ynew[:], in0=t2[:], scalar=1.5, in1=y[:],
            op0=ALU.add, op1=ALU.mult)                # y*(1.5 - 0.5 ss y^2)
        y = ynew if not last else None


def emit_normalize_transpose(c, g):
    """Scale group-g rows to unit norm (bf16), compute pos dots,
    PE-transpose into PSUM collectors and copy to the bf16 B_T tiles."""
    nc = c.nc
    psA = c.psum_pool.tile([P, c.chunk], BF16, tag="ps", name="psA")
    psB = c.psum_pool.tile([P, c.chunk], BF16, tag="ps", name="psB")
    for t in range(c.TPG):
        ti = g * c.TPG + t
        if ti < c.MT:
            nrm = c.keep_pool.tile([P, H], BF16, tag="keep", name=f"keep_{ti}")
        else:
            nrm = c.nrm_pool.tile([P, H], BF16, tag="nrm", name="nrm")
        nc.vector.tensor_scalar_mul(nrm[:], c.raws[g][t][:],
                                    c.inv_all[:, ti:ti + 1])
        if ti < c.MT:
            c.kept[ti] = nrm
        if c.half <= ti < c.half + c.MT:
            m = ti - c.half
            sq2 = c.sq_pool.tile([P, H], F32, tag="sq2", name="sq2")
            nc.vector.scalar_tensor_tensor(
                out=sq2[:], in0=nrm[:], scalar=1.0, in1=c.kept[m][:],
                op0=ALU.mult, op1=ALU.mult,
                accum_out=c.pos_all[:, m:m + 1],
            )
        nc.tensor.transpose(psA[:, t * P:(t + 1) * P], nrm[:, 0:P],
                            c.identity[:])
        nc.tensor.transpose(psB[:, t * P:(t + 1) * P], nrm[:, P:2 * P],
                            c.identity[:])
    del c.raws[g]
    # copy PSUM collectors into B_T (bf16, DVE 2x mode)
    nc.vector.tensor_copy(out=c.BT0s[g][:], in_=psA[:])
    nc.vector.tensor_copy(out=c.BT1s[g][:], in_=psB[:])


def emit_rep(c, rep):
    nc = c.nc
    c.kept = [None] * c.MT
    c.raws = {}

    emit_loads(c, 0)
    emit_normalize_transpose(c, 0)
    if c.G > 1:
        emit_loads(c, 1)

    for g in range(c.G):
        for m in range(c.MT):
            ps = c.psum_pool.tile([P, c.chunk], F32, tag="ps", name="mm_ps")
            lhs0 = c.BT0s[0][:, m * P:(m + 1) * P]
            lhs1 = c.BT1s[0][:, m * P:(m + 1) * P]
            for j in range(c.NJ):
                nc.tensor.matmul(
                    ps[:, j * 512:(j + 1) * 512], lhs0,
                    c.BT0s[g][:, j * 512:(j + 1) * 512],
                    start=True, stop=False)
            for j in range(c.NJ):
                nc.tensor.matmul(
                    ps[:, j * 512:(j + 1) * 512], lhs1,
                    c.BT1s[g][:, j * 512:(j + 1) * 512],
                    start=False, stop=True)
            # exp values are dead; only accum_out matters.  bf16 SBUF out
            # hits the ACT 2x output mode (vs 1x for f32-in-place), keeping
            # ACT faster than PE so the PE never micro-idles (HAM warm).
            acc = c.rs_all[:, m * c.G + g: m * c.G + g + 1]
            if c.exp_sbuf:
                eo = c.exp_pool.tile([P, c.chunk], BF16, tag="eo", name="eo")
                nc.scalar.activation(
                    out=eo[:], in_=ps[:], func=AF.Exp, scale=2.0, accum_out=acc)
            else:
                nc.scalar.activation(
                    out=ps[:], in_=ps[:], func=AF.Exp, scale=2.0, accum_out=acc)
            # interleave next group's prologue into this group's mm stream
            if m == c.NT_AT and g + 1 < c.G:
                emit_normalize_transpose(c, g + 1)
            if m == c.LD_AT and g + 2 < c.G:
                emit_loads(c, g + 2)

    # ---- finalize ----
    MT, G = c.MT, c.G
    rowsum = c.fin_pool.tile([P, MT], F32, tag="rowsum", name="rowsum")
    nc.vector.tensor_reduce(
        out=rowsum[:], in_=c.rs_all[:].rearrange("p (m g) -> p m g", g=G),
        axis=mybir.AxisListType.X, op=ALU.add)
    denom = c.fin_pool.tile([P, MT], F32, tag="denom", name="denom")
    nc.vector.tensor_scalar_add(denom[:], rowsum[:], -float(np.exp(2.0)))
    lnd = c.fin_pool.tile([P, MT], F32, tag="lnd", name="lnd")
    nc.scalar.activation(out=lnd[:], in_=denom[:], func=AF.Ln)
    ntv = c.fin_pool.tile([P, MT], F32, tag="ntv", name="ntv")
    # nt = (pos * -2) + ln(denom)
    nc.vector.scalar_tensor_tensor(
        out=ntv[:], in0=c.pos_all[:], scalar=-2.0, in1=lnd[:],
        op0=ALU.mult, op1=ALU.add)
    nc.sync.dma_start(
        out=c.nt_dram.ap()[:, rep * MT:(rep + 1) * MT], in_=ntv[:])


_CACHE = {}


def _get_program():
    if "nc" not in _CACHE:
        _CACHE["nc"] = build_program()
    return _CACHE["nc"]


def kernel(x: np.ndarray, y: np.ndarray) -> np.ndarray:
    x = np.asarray(x, dtype=np.float32)
    y = np.asarray(y, dtype=np.float32)
    xy = np.concatenate([x, y], axis=0)          # [16384, 256]

    nc, in_name, out_name = _get_program()

    in_maps = []
    for c in range(N_CORES):
        off = c * N_MINE
        b_rot = np.ascontiguousarray(np.roll(xy, -off, axis=0))
        in_maps.append({in_name: b_rot})

    res = bass_utils.run_bass_kernel_spmd(
        nc, in_maps, core_ids=list(range(N_CORES)))

    # nt[c][p, m] = loss for global row (c*N_MINE + m*128 + p)
    rows = np.concatenate(
        [res.results[c][out_name].T.reshape(-1) for c in range(N_CORES)])
    loss = rows.astype(np.float64).mean()
    return np.float32(loss)

